# revision 5
# baseline (speedup 1.0000x reference)
"""GRU cell (EncoderRNN single step) on 8 Trainium2 NeuronCores.

Full inputs -> full output. Sharding: each core owns a 256-wide slice of the
hidden dimension across all three gates (rows of w_ih/w_hh), so there are no
collectives; the host gathers the embedding row, broadcasts x/h, and
concatenates the 8 per-core h_new slices.

Device compute per core: 12x fused multiply+reduce (tensor_tensor_reduce) on
the vector engine over [128, 2048] fp32 weight tiles (natural layout, K along
the free axis, one output row per partition), bias folded in as the reduce
initial value, then the GRU gate elementwise math on [128, 2] tiles.
"""

import sys

if "/opt/trn_rl_repo" not in sys.path:
    sys.path.insert(0, "/opt/trn_rl_repo")

import numpy as np

H = 2048
NCORES = 8
HC = H // NCORES          # 256 hidden elems per core
WT = (3 * HC) // 128      # 6 weight tiles of 128 rows per core
UT = HC // 128            # 2 columns for the per-core [128, 2] gate slices

_CACHE = {}


def _build():
    """Build the (single) SPMD Bass program under Tile. Same program on all
    cores; per-core data differences live entirely in the input maps."""
    from concourse import bacc, tile, mybir

    f32 = mybir.dt.float32
    Alu = mybir.AluOpType
    Act = mybir.ActivationFunctionType

    # Bacc (not raw Bass): its compile() runs generate_event_semaphores,
    # which legalizes Tile's multi-wait instructions down to the 1 sync
    # wait per instruction this walrus build accepts.
    nc = bacc.Bacc("TRN2", target_bir_lowering=False, debug=False, num_devices=NCORES)

    wih = nc.dram_tensor("wih", [3 * HC, H], f32, kind="ExternalInput")
    whh = nc.dram_tensor("whh", [3 * HC, H], f32, kind="ExternalInput")
    xb = nc.dram_tensor("xb", [128, H], f32, kind="ExternalInput")
    hb = nc.dram_tensor("hb", [128, H], f32, kind="ExternalInput")
    bih = nc.dram_tensor("bih", [128, WT], f32, kind="ExternalInput")
    bhh = nc.dram_tensor("bhh", [128, WT], f32, kind="ExternalInput")
    hs = nc.dram_tensor("hs", [128, UT], f32, kind="ExternalInput")
    hout = nc.dram_tensor("hout", [128, UT], f32, kind="ExternalOutput")

    with tile.TileContext(nc) as tc:
        with (
            tc.tile_pool(name="consts", bufs=1) as cp,
            tc.tile_pool(name="w", bufs=4) as wp,
        ):
            xb_t = cp.tile([128, H], f32, name="xb_t")
            nc.sync.dma_start(out=xb_t[:, :], in_=xb.ap()[:, :])
            hb_t = cp.tile([128, H], f32, name="hb_t")
            nc.sync.dma_start(out=hb_t[:, :], in_=hb.ap()[:, :])
            bih_t = cp.tile([128, WT], f32, name="bih_t")
            nc.sync.dma_start(out=bih_t[:, :], in_=bih.ap()[:, :])
            bhh_t = cp.tile([128, WT], f32, name="bhh_t")
            nc.sync.dma_start(out=bhh_t[:, :], in_=bhh.ap()[:, :])
            hs_t = cp.tile([128, UT], f32, name="hs_t")
            nc.sync.dma_start(out=hs_t[:, :], in_=hs.ap()[:, :])

            gi_t = cp.tile([128, WT], f32, name="gi_t")
            gh_t = cp.tile([128, WT], f32, name="gh_t")
            scratch = cp.tile([128, H], f32, name="ttr_scratch")

            for t in range(WT):
                for (wdram, vec_t, acc_t) in (
                    (wih, xb_t, gi_t),
                    (whh, hb_t, gh_t),
                ):
                    w_t = wp.tile([128, H], f32, tag="w")
                    nc.sync.dma_start(
                        out=w_t[:, :], in_=wdram.ap()[t * 128 : (t + 1) * 128, :]
                    )
                    # Fused row-dot on DVE: accum_out[p] = sum_k w[p,k]*vec[p,k]
                    nc.vector.scalar_tensor_tensor(
                        out=scratch[:, :],
                        in0=w_t[:, :],
                        scalar=0.0,
                        in1=vec_t[:, :],
                        op0=Alu.bypass,
                        op1=Alu.mult,
                        accum_out=acc_t[:, t : t + 1],
                    )
            nc.vector.tensor_tensor(
                out=gi_t[:, :], in0=gi_t[:, :], in1=bih_t[:, :], op=Alu.add
            )
            nc.vector.tensor_tensor(
                out=gh_t[:, :], in0=gh_t[:, :], in1=bhh_t[:, :], op=Alu.add
            )

            # GRU gate math on [128, UT] slices. Column layout of gi/gh:
            # [0:UT] = r rows, [UT:2UT] = z rows, [2UT:3UT] = n rows.
            r_t = cp.tile([128, UT], f32, name="r_t")
            z_t = cp.tile([128, UT], f32, name="z_t")
            n_t = cp.tile([128, UT], f32, name="n_t")
            tmp = cp.tile([128, UT], f32, name="tmp")
            hnew = cp.tile([128, UT], f32, name="hnew")

            u0, u1, u2 = 0, UT, 2 * UT
            nc.vector.tensor_tensor(
                out=tmp[:, :], in0=gi_t[:, u0:u1], in1=gh_t[:, u0:u1], op=Alu.add
            )
            nc.scalar.activation(out=r_t[:, :], in_=tmp[:, :], func=Act.Sigmoid)
            nc.vector.tensor_tensor(
                out=tmp[:, :], in0=gi_t[:, u1:u2], in1=gh_t[:, u1:u2], op=Alu.add
            )
            nc.scalar.activation(out=z_t[:, :], in_=tmp[:, :], func=Act.Sigmoid)
            # n = tanh(i_n + r * h_n)
            nc.vector.tensor_tensor(
                out=tmp[:, :], in0=r_t[:, :], in1=gh_t[:, u2 : u2 + UT], op=Alu.mult
            )
            nc.vector.tensor_tensor(
                out=tmp[:, :], in0=gi_t[:, u2 : u2 + UT], in1=tmp[:, :], op=Alu.add
            )
            nc.scalar.activation(out=n_t[:, :], in_=tmp[:, :], func=Act.Tanh)
            # h_new = n + z * (h - n)
            nc.vector.tensor_tensor(
                out=tmp[:, :], in0=hs_t[:, :], in1=n_t[:, :], op=Alu.subtract
            )
            nc.vector.tensor_tensor(
                out=tmp[:, :], in0=z_t[:, :], in1=tmp[:, :], op=Alu.mult
            )
            nc.vector.tensor_tensor(
                out=hnew[:, :], in0=n_t[:, :], in1=tmp[:, :], op=Alu.add
            )
            nc.sync.dma_start(out=hout.ap()[:, :], in_=hnew[:, :])

    nc.compile()
    return nc


def get_nc():
    if "nc" not in _CACHE:
        _CACHE["nc"] = _build()
    return _CACHE["nc"]


def make_in_maps(inputs):
    """Host-side sharding: full-input dict -> 8 per-core input maps."""
    emb = np.ascontiguousarray(np.asarray(inputs["emb"], dtype=np.float32))
    w_ih = np.asarray(inputs["w_ih"], dtype=np.float32)
    w_hh = np.asarray(inputs["w_hh"], dtype=np.float32)
    b_ih = np.asarray(inputs["b_ih"], dtype=np.float32)
    b_hh = np.asarray(inputs["b_hh"], dtype=np.float32)
    idx = int(np.asarray(inputs["input"]).reshape(-1)[0])
    x = emb[idx]
    h = np.asarray(inputs["hidden"], dtype=np.float32).reshape(H)

    xb = np.ascontiguousarray(np.broadcast_to(x, (128, H)))
    hb = np.ascontiguousarray(np.broadcast_to(h, (128, H)))

    in_maps = []
    for c in range(NCORES):
        rows = [slice(g * H + c * HC, g * H + (c + 1) * HC) for g in range(3)]
        wih_c = np.ascontiguousarray(np.concatenate([w_ih[s] for s in rows], axis=0))
        whh_c = np.ascontiguousarray(np.concatenate([w_hh[s] for s in rows], axis=0))
        bih_c = np.ascontiguousarray(
            np.concatenate([b_ih[s] for s in rows]).reshape(WT, 128).T
        )
        bhh_c = np.ascontiguousarray(
            np.concatenate([b_hh[s] for s in rows]).reshape(WT, 128).T
        )
        hs_c = np.ascontiguousarray(h[c * HC : (c + 1) * HC].reshape(UT, 128).T)
        in_maps.append(
            {
                "wih": wih_c,
                "whh": whh_c,
                "xb": xb,
                "hb": hb,
                "bih": bih_c,
                "bhh": bhh_c,
                "hs": hs_c,
            }
        )
    return in_maps


def run_on_hw(in_maps, trace=False):
    from concourse.bass_utils import run_bass_kernel_spmd

    kwargs = {}
    if trace:
        kwargs.update(trace=True, trace_cores=list(range(NCORES)))
    return run_bass_kernel_spmd(get_nc(), in_maps, core_ids=list(range(NCORES)), **kwargs)


def assemble(results):
    h_new = np.concatenate(
        [np.ascontiguousarray(results[c]["hout"].T).reshape(HC) for c in range(NCORES)]
    )
    out = h_new.reshape(1, 1, H).astype(np.float32)
    return out, out.copy()


def kernel(**inputs):
    in_maps = make_in_maps(inputs)
    res = run_on_hw(in_maps)
    return assemble(res.results)


# revision 10
# speedup vs baseline: 1.0891x; 1.0891x over previous
"""GRU cell (EncoderRNN single step) on 8 Trainium2 NeuronCores.

Full inputs -> full output. Sharding: each core owns a 256-wide slice of the
hidden dimension across all three gates (rows of w_ih/w_hh), so there are no
collectives; the host gathers the embedding row, broadcasts x/h, and
concatenates the 8 per-core h_new slices.

Device compute per core: 12x fused multiply+reduce (tensor_tensor_reduce) on
the vector engine over [128, 2048] fp32 weight tiles (natural layout, K along
the free axis, one output row per partition), bias folded in as the reduce
initial value, then the GRU gate elementwise math on [128, 2] tiles.
"""

import sys

if "/opt/trn_rl_repo" not in sys.path:
    sys.path.insert(0, "/opt/trn_rl_repo")

import numpy as np

H = 2048
NCORES = 8
HC = H // NCORES          # 256 hidden elems per core
WT = (3 * HC) // 128      # 6 weight tiles of 128 rows per core
UT = HC // 128            # 2 columns for the per-core [128, 2] gate slices

_CACHE = {}


def _build():
    """Build the (single) SPMD Bass program under Tile. Same program on all
    cores; per-core data differences live entirely in the input maps."""
    from concourse import bacc, tile, mybir

    f32 = mybir.dt.float32
    Alu = mybir.AluOpType
    Act = mybir.ActivationFunctionType

    # Bacc (not raw Bass): its compile() runs generate_event_semaphores,
    # which legalizes Tile's multi-wait instructions down to the 1 sync
    # wait per instruction this walrus build accepts.
    nc = bacc.Bacc("TRN2", target_bir_lowering=False, debug=False, num_devices=NCORES)

    wih = nc.dram_tensor("wih", [3 * HC, H], f32, kind="ExternalInput")
    whh = nc.dram_tensor("whh", [3 * HC, H], f32, kind="ExternalInput")
    xh = nc.dram_tensor("xh", [2, H], f32, kind="ExternalInput")
    bih = nc.dram_tensor("bih", [128, WT], f32, kind="ExternalInput")
    bhh = nc.dram_tensor("bhh", [128, WT], f32, kind="ExternalInput")
    hs = nc.dram_tensor("hs", [128, UT], f32, kind="ExternalInput")
    hout = nc.dram_tensor("hout", [128, UT], f32, kind="ExternalOutput")

    with tile.TileContext(nc) as tc:
        with (
            tc.tile_pool(name="consts", bufs=1) as cp,
            tc.tile_pool(name="w", bufs=4) as wp,
        ):
            xrow_t = cp.tile([1, H], f32, name="xrow_t")
            nc.sync.dma_start(out=xrow_t[:, :], in_=xh.ap()[0:1, :])
            hrow_t = cp.tile([1, H], f32, name="hrow_t")
            nc.sync.dma_start(out=hrow_t[:, :], in_=xh.ap()[1:2, :])
            xb_t = cp.tile([128, H], f32, name="xb_t")
            nc.gpsimd.partition_broadcast(xb_t[:, :], xrow_t[0:1, :])
            hb_t = cp.tile([128, H], f32, name="hb_t")
            nc.gpsimd.partition_broadcast(hb_t[:, :], hrow_t[0:1, :])
            bih_t = cp.tile([128, WT], f32, name="bih_t")
            nc.sync.dma_start(out=bih_t[:, :], in_=bih.ap()[:, :])
            bhh_t = cp.tile([128, WT], f32, name="bhh_t")
            nc.sync.dma_start(out=bhh_t[:, :], in_=bhh.ap()[:, :])
            hs_t = cp.tile([128, UT], f32, name="hs_t")
            nc.sync.dma_start(out=hs_t[:, :], in_=hs.ap()[:, :])

            gi_t = cp.tile([128, WT], f32, name="gi_t")
            gh_t = cp.tile([128, WT], f32, name="gh_t")
            scratch = cp.tile([128, H], f32, name="ttr_scratch")

            for t in range(WT):
                for (wdram, vec_t, acc_t) in (
                    (wih, xb_t, gi_t),
                    (whh, hb_t, gh_t),
                ):
                    w_t = wp.tile([128, H], f32, tag="w")
                    nc.sync.dma_start(
                        out=w_t[:, :], in_=wdram.ap()[t * 128 : (t + 1) * 128, :]
                    )
                    # Fused row-dot on DVE: accum_out[p] = sum_k w[p,k]*vec[p,k]
                    nc.vector.scalar_tensor_tensor(
                        out=scratch[:, :],
                        in0=w_t[:, :],
                        scalar=0.0,
                        in1=vec_t[:, :],
                        op0=Alu.bypass,
                        op1=Alu.mult,
                        accum_out=acc_t[:, t : t + 1],
                    )
            nc.vector.tensor_tensor(
                out=gi_t[:, :], in0=gi_t[:, :], in1=bih_t[:, :], op=Alu.add
            )
            nc.vector.tensor_tensor(
                out=gh_t[:, :], in0=gh_t[:, :], in1=bhh_t[:, :], op=Alu.add
            )

            # GRU gate math on [128, UT] slices. Column layout of gi/gh:
            # [0:UT] = r rows, [UT:2UT] = z rows, [2UT:3UT] = n rows.
            r_t = cp.tile([128, UT], f32, name="r_t")
            z_t = cp.tile([128, UT], f32, name="z_t")
            n_t = cp.tile([128, UT], f32, name="n_t")
            tmp = cp.tile([128, UT], f32, name="tmp")
            hnew = cp.tile([128, UT], f32, name="hnew")

            u0, u1, u2 = 0, UT, 2 * UT
            nc.vector.tensor_tensor(
                out=tmp[:, :], in0=gi_t[:, u0:u1], in1=gh_t[:, u0:u1], op=Alu.add
            )
            nc.scalar.activation(out=r_t[:, :], in_=tmp[:, :], func=Act.Sigmoid)
            nc.vector.tensor_tensor(
                out=tmp[:, :], in0=gi_t[:, u1:u2], in1=gh_t[:, u1:u2], op=Alu.add
            )
            nc.scalar.activation(out=z_t[:, :], in_=tmp[:, :], func=Act.Sigmoid)
            # n = tanh(i_n + r * h_n)
            nc.vector.tensor_tensor(
                out=tmp[:, :], in0=r_t[:, :], in1=gh_t[:, u2 : u2 + UT], op=Alu.mult
            )
            nc.vector.tensor_tensor(
                out=tmp[:, :], in0=gi_t[:, u2 : u2 + UT], in1=tmp[:, :], op=Alu.add
            )
            nc.scalar.activation(out=n_t[:, :], in_=tmp[:, :], func=Act.Tanh)
            # h_new = n + z * (h - n)
            nc.vector.tensor_tensor(
                out=tmp[:, :], in0=hs_t[:, :], in1=n_t[:, :], op=Alu.subtract
            )
            nc.vector.tensor_tensor(
                out=tmp[:, :], in0=z_t[:, :], in1=tmp[:, :], op=Alu.mult
            )
            nc.vector.tensor_tensor(
                out=hnew[:, :], in0=n_t[:, :], in1=tmp[:, :], op=Alu.add
            )
            nc.sync.dma_start(out=hout.ap()[:, :], in_=hnew[:, :])

    nc.compile()
    return nc


def get_nc():
    if "nc" not in _CACHE:
        _CACHE["nc"] = _build()
    return _CACHE["nc"]


def make_in_maps(inputs):
    """Host-side sharding: full-input dict -> 8 per-core input maps."""
    emb = np.ascontiguousarray(np.asarray(inputs["emb"], dtype=np.float32))
    w_ih = np.asarray(inputs["w_ih"], dtype=np.float32)
    w_hh = np.asarray(inputs["w_hh"], dtype=np.float32)
    b_ih = np.asarray(inputs["b_ih"], dtype=np.float32)
    b_hh = np.asarray(inputs["b_hh"], dtype=np.float32)
    idx = int(np.asarray(inputs["input"]).reshape(-1)[0])
    x = emb[idx]
    h = np.asarray(inputs["hidden"], dtype=np.float32).reshape(H)

    xh_host = np.ascontiguousarray(np.stack([x, h], axis=0))

    in_maps = []
    for c in range(NCORES):
        rows = [slice(g * H + c * HC, g * H + (c + 1) * HC) for g in range(3)]
        wih_c = np.ascontiguousarray(np.concatenate([w_ih[s] for s in rows], axis=0))
        whh_c = np.ascontiguousarray(np.concatenate([w_hh[s] for s in rows], axis=0))
        bih_c = np.ascontiguousarray(
            np.concatenate([b_ih[s] for s in rows]).reshape(WT, 128).T
        )
        bhh_c = np.ascontiguousarray(
            np.concatenate([b_hh[s] for s in rows]).reshape(WT, 128).T
        )
        hs_c = np.ascontiguousarray(h[c * HC : (c + 1) * HC].reshape(UT, 128).T)
        in_maps.append(
            {
                "wih": wih_c,
                "whh": whh_c,
                "xh": xh_host,
                "bih": bih_c,
                "bhh": bhh_c,
                "hs": hs_c,
            }
        )
    return in_maps


def run_on_hw(in_maps, trace=False):
    from concourse.bass_utils import run_bass_kernel_spmd

    kwargs = {}
    if trace:
        kwargs.update(trace=True, trace_cores=list(range(NCORES)))
    return run_bass_kernel_spmd(get_nc(), in_maps, core_ids=list(range(NCORES)), **kwargs)


def assemble(results):
    h_new = np.concatenate(
        [np.ascontiguousarray(results[c]["hout"].T).reshape(HC) for c in range(NCORES)]
    )
    out = h_new.reshape(1, 1, H).astype(np.float32)
    return out, out.copy()


def kernel(**inputs):
    in_maps = make_in_maps(inputs)
    res = run_on_hw(in_maps)
    return assemble(res.results)


# revision 11
# speedup vs baseline: 1.1728x; 1.0769x over previous
"""GRU cell (EncoderRNN single step) on 8 Trainium2 NeuronCores.

Full inputs -> full output. Sharding: each core owns a 256-wide slice of the
hidden dimension across all three gates (rows of w_ih/w_hh), so there are no
collectives; the host gathers the embedding row (only that row of the table is
ever needed) and concatenates the 8 per-core h_new slices.

Device compute per core:
- x and h rows (8KB each) are DMA'd once and broadcast to all 128 partitions
  by a PE ones-matmul into PSUM (bit-exact for fp32), 4 banks per vector.
- 12x fused multiply+reduce (scalar_tensor_tensor, op0=bypass op1=mult with
  accum_out) on the vector engine: each op row-dots a [128, 2048] fp32 weight
  tile (natural layout, one output row per partition) against the broadcast
  vector read straight from PSUM.
- Weight tiles stream over the SP HWDGE ring (12 x 1MB, bufs=12 so DMA never
  stalls); small tensors ride the ACT HWDGE ring.
- STT order: w_ih r/z tiles, w_hh r/z, w_hh n, w_ih n - so the r/z sigmoid
  and r*(h_n+b) products overlap the tail of the weight stream and the
  critical path after the last weight tile is short.
"""

import sys

if "/opt/trn_rl_repo" not in sys.path:
    sys.path.insert(0, "/opt/trn_rl_repo")

import numpy as np

H = 2048
NCORES = 8
HC = H // NCORES          # 256 hidden elems per core
WT = (3 * HC) // 128      # 6 weight tiles of 128 rows per core
UT = HC // 128            # 2 columns for the per-core [128, 2] gate slices

_CACHE = {}


def _build():
    from concourse import bacc, tile, mybir

    f32 = mybir.dt.float32
    Alu = mybir.AluOpType
    Act = mybir.ActivationFunctionType

    # Bacc (not raw Bass): its compile() runs generate_event_semaphores,
    # which legalizes Tile's multi-wait instructions down to the 1 sync
    # wait per instruction this walrus build accepts.
    nc = bacc.Bacc("TRN2", target_bir_lowering=False, debug=False, num_devices=NCORES)

    wih = nc.dram_tensor("wih", [3 * HC, H], f32, kind="ExternalInput")
    whh = nc.dram_tensor("whh", [3 * HC, H], f32, kind="ExternalInput")
    xh = nc.dram_tensor("xh", [2, H], f32, kind="ExternalInput")
    brz = nc.dram_tensor("brz", [128, 2 * UT], f32, kind="ExternalInput")
    bin_ = nc.dram_tensor("bin", [128, UT], f32, kind="ExternalInput")
    bhn = nc.dram_tensor("bhn", [128, UT], f32, kind="ExternalInput")
    hs = nc.dram_tensor("hs", [128, UT], f32, kind="ExternalInput")
    hout = nc.dram_tensor("hout", [128, UT], f32, kind="ExternalOutput")

    with tile.TileContext(nc) as tc:
        with (
            tc.tile_pool(name="consts", bufs=1) as cp,
            tc.tile_pool(name="w", bufs=12) as wp,
            tc.tile_pool(name="bc", bufs=1, space="PSUM") as bp,
        ):
            # Small inputs ride the ACT HWDGE ring; SP ring is weights-only.
            xrow_t = cp.tile([1, H], f32, name="xrow_t")
            nc.scalar.dma_start(out=xrow_t[:, :], in_=xh.ap()[0:1, :])
            hrow_t = cp.tile([1, H], f32, name="hrow_t")
            nc.scalar.dma_start(out=hrow_t[:, :], in_=xh.ap()[1:2, :])
            brz_t = cp.tile([128, 2 * UT], f32, name="brz_t")
            nc.scalar.dma_start(out=brz_t[:, :], in_=brz.ap()[:, :])
            bin_t = cp.tile([128, UT], f32, name="bin_t")
            nc.scalar.dma_start(out=bin_t[:, :], in_=bin_.ap()[:, :])
            bhn_t = cp.tile([128, UT], f32, name="bhn_t")
            nc.scalar.dma_start(out=bhn_t[:, :], in_=bhn.ap()[:, :])
            hs_t = cp.tile([128, UT], f32, name="hs_t")
            nc.scalar.dma_start(out=hs_t[:, :], in_=hs.ap()[:, :])

            # Broadcast x and h across partitions: ones-matmul into PSUM.
            ones = cp.tile([1, 128], f32, name="ones")
            nc.vector.memset(ones[:, :], 1.0)
            xb_t = bp.tile([128, H], f32, name="xb_t")   # PSUM banks 0-3
            hb_t = bp.tile([128, H], f32, name="hb_t")   # PSUM banks 4-7
            for j in range(H // 512):
                nc.tensor.matmul(
                    xb_t[:, j * 512 : (j + 1) * 512],
                    lhsT=ones[0:1, :],
                    rhs=xrow_t[0:1, j * 512 : (j + 1) * 512],
                    start=True,
                    stop=True,
                )
            for j in range(H // 512):
                nc.tensor.matmul(
                    hb_t[:, j * 512 : (j + 1) * 512],
                    lhsT=ones[0:1, :],
                    rhs=hrow_t[0:1, j * 512 : (j + 1) * 512],
                    start=True,
                    stop=True,
                )

            girz = cp.tile([128, 2 * UT], f32, name="girz")
            ghrz = cp.tile([128, 2 * UT], f32, name="ghrz")
            gin = cp.tile([128, UT], f32, name="gin")
            ghn = cp.tile([128, UT], f32, name="ghn")
            scratch = cp.tile([128, H], f32, name="stt_scratch")

            # (dram, row-offset tile idx, broadcast vec, accum tile, accum col)
            schedule = (
                [(wih, t, xb_t, girz, t) for t in range(2 * UT)]
                + [(whh, t, hb_t, ghrz, t) for t in range(2 * UT)]
                + [(whh, 2 * UT + u, hb_t, ghn, u) for u in range(UT)]
                + [(wih, 2 * UT + u, xb_t, gin, u) for u in range(UT)]
            )
            for (wdram, t, vec_t, acc_t, col) in schedule:
                w_t = wp.tile([128, H], f32, tag="w")
                nc.sync.dma_start(
                    out=w_t[:, :], in_=wdram.ap()[t * 128 : (t + 1) * 128, :]
                )
                # Fused row-dot on DVE: accum_out[p] = sum_k w[p,k]*vec[p,k]
                nc.vector.scalar_tensor_tensor(
                    out=scratch[:, :],
                    in0=w_t[:, :],
                    scalar=0.0,
                    in1=vec_t[:, :],
                    op0=Alu.bypass,
                    op1=Alu.mult,
                    accum_out=acc_t[:, col : col + 1],
                )

            # GRU gate math on [128, UT] column slices (partition-major).
            rzp = cp.tile([128, 2 * UT], f32, name="rzp")
            rz = cp.tile([128, 2 * UT], f32, name="rz")
            hnb = cp.tile([128, UT], f32, name="hnb")
            t3 = cp.tile([128, UT], f32, name="t3")
            t4 = cp.tile([128, UT], f32, name="t4")
            n_t = cp.tile([128, UT], f32, name="n_t")
            t5 = cp.tile([128, UT], f32, name="t5")
            hnew = cp.tile([128, UT], f32, name="hnew")

            nc.vector.tensor_tensor(out=rzp[:, :], in0=girz[:, :], in1=ghrz[:, :], op=Alu.add)
            nc.vector.tensor_tensor(out=rzp[:, :], in0=rzp[:, :], in1=brz_t[:, :], op=Alu.add)
            nc.scalar.activation(out=rz[:, :], in_=rzp[:, :], func=Act.Sigmoid)
            nc.vector.tensor_tensor(out=hnb[:, :], in0=ghn[:, :], in1=bhn_t[:, :], op=Alu.add)
            nc.vector.tensor_tensor(out=t3[:, :], in0=rz[:, 0:UT], in1=hnb[:, :], op=Alu.mult)
            nc.vector.tensor_tensor(out=t4[:, :], in0=gin[:, :], in1=bin_t[:, :], op=Alu.add)
            nc.vector.tensor_tensor(out=t4[:, :], in0=t4[:, :], in1=t3[:, :], op=Alu.add)
            nc.scalar.activation(out=n_t[:, :], in_=t4[:, :], func=Act.Tanh)
            # h_new = n + z * (h - n)
            nc.vector.tensor_tensor(out=t5[:, :], in0=hs_t[:, :], in1=n_t[:, :], op=Alu.subtract)
            nc.vector.tensor_tensor(out=t5[:, :], in0=rz[:, UT : 2 * UT], in1=t5[:, :], op=Alu.mult)
            nc.vector.tensor_tensor(out=hnew[:, :], in0=n_t[:, :], in1=t5[:, :], op=Alu.add)
            nc.sync.dma_start(out=hout.ap()[:, :], in_=hnew[:, :])

    nc.compile()
    return nc


def get_nc():
    if "nc" not in _CACHE:
        _CACHE["nc"] = _build()
    return _CACHE["nc"]


def make_in_maps(inputs):
    """Host-side sharding: full-input dict -> 8 per-core input maps."""
    emb = np.asarray(inputs["emb"], dtype=np.float32)
    w_ih = np.asarray(inputs["w_ih"], dtype=np.float32)
    w_hh = np.asarray(inputs["w_hh"], dtype=np.float32)
    b_ih = np.asarray(inputs["b_ih"], dtype=np.float32)
    b_hh = np.asarray(inputs["b_hh"], dtype=np.float32)
    idx = int(np.asarray(inputs["input"]).reshape(-1)[0])
    x = np.ascontiguousarray(emb[idx])
    h = np.asarray(inputs["hidden"], dtype=np.float32).reshape(H)

    xh_host = np.ascontiguousarray(np.stack([x, h], axis=0))
    bsum = b_ih + b_hh

    in_maps = []
    for c in range(NCORES):
        sl = [slice(g * H + c * HC, g * H + (c + 1) * HC) for g in range(3)]
        wih_c = np.ascontiguousarray(np.concatenate([w_ih[s] for s in sl], axis=0))
        whh_c = np.ascontiguousarray(np.concatenate([w_hh[s] for s in sl], axis=0))
        brz_c = np.ascontiguousarray(
            np.concatenate([bsum[sl[0]], bsum[sl[1]]]).reshape(2 * UT, 128).T
        )
        bin_c = np.ascontiguousarray(b_ih[sl[2]].reshape(UT, 128).T)
        bhn_c = np.ascontiguousarray(b_hh[sl[2]].reshape(UT, 128).T)
        hs_c = np.ascontiguousarray(h[c * HC : (c + 1) * HC].reshape(UT, 128).T)
        in_maps.append(
            {
                "wih": wih_c,
                "whh": whh_c,
                "xh": xh_host,
                "brz": brz_c,
                "bin": bin_c,
                "bhn": bhn_c,
                "hs": hs_c,
            }
        )
    return in_maps


def run_on_hw(in_maps, trace=False):
    from concourse.bass_utils import run_bass_kernel_spmd

    kwargs = {}
    if trace:
        kwargs.update(trace=True, trace_cores=list(range(NCORES)))
    return run_bass_kernel_spmd(get_nc(), in_maps, core_ids=list(range(NCORES)), **kwargs)


def assemble(results):
    h_new = np.concatenate(
        [np.ascontiguousarray(results[c]["hout"].T).reshape(HC) for c in range(NCORES)]
    )
    out = h_new.reshape(1, 1, H).astype(np.float32)
    return out, out.copy()


def kernel(**inputs):
    in_maps = make_in_maps(inputs)
    res = run_on_hw(in_maps)
    return assemble(res.results)
